# revision 19
# baseline (speedup 1.0000x reference)
"""ComplEx scoring kernel for 8 Trainium2 NeuronCores.

Math: score[b, e] = Re(<h_b * r_b, conj(ent_e)>) with h = ent_emb[triples[:,0]],
r = rel_emb[triples[:,1]].  Writing ans_b = concat(re_h*re_r - im_h*im_r,
re_h*im_r + im_h*re_r) (shape [B, 512]), the score is exactly
score = ans @ ent_emb.T  — one [1024, 512] x [512, 200000] GEMM.

Strategy (vocab/tensor parallel on the entity axis, 25000 entities/core,
padded to 25088 = 49x512 columns):
  - host: tiny gather + complex multiply -> ans  (microseconds)
  - the GEMM is TensorE-bound (26.3 GFLOP/core vs 78.6 TF/s bf16), so the
    entity axis is split into a bf16 part (37 tiles of 512) and an fp8-e4m3
    DoubleRow part (12 tiles) that runs the PE at 2 MACs/cell/cycle.  The
    fp8 fraction (24% of entities) is sized so the fp8 quantization noise
    (3.75e-2 on those columns) keeps the global rel err at ~1.85e-2 (<2e-2).
  - per core: score_bf16[1024, 18944] + score_fp8[1024, 6144], both f16.
    PE is pre-warmed with dummy matmuls so the HAM clock gate opens during
    the preamble/first DMAs instead of 3.4us into real work.  Inputs ride
    the ACT HWDGE ring, outputs the SP ring.  The fp8 section runs second
    to last: its score writes come at 2x the bf16 rate (~300 GB/s), and the
    small final bf16 group absorbs the write backlog so the kernel tail is
    just the last small DMA's completion receipt.
  - host: concatenate the 8 column slabs, per-region unscale, drop padding
"""

import numpy as np
import ml_dtypes

NCORES = 8
NUM_ENT = 200000
EMB = 512
B = 1024
SHARD = NUM_ENT // NCORES      # 25000 entities per core
NTILE = 512                    # matmul moving free dim == one PSUM bank
NB_TILES = 37                  # bf16 512-tiles per core
NF_TILES = 12                  # fp8 512-tiles per core
NB = NB_TILES * NTILE          # 18944 bf16 columns
NF = NF_TILES * NTILE          # 6144 fp8 columns (6056 real + 88 pad)
SHARD_PAD = NB + NF            # 25088
NF_REAL = SHARD - NB           # 6056 real entities in the fp8 region
GROUPS = [4, 3, 7, 7, 7, 7, 2] # bf16 tile groups (DMA/reuse granularity)
GN_FULL = 7 * NTILE            # 3584
KCH = EMB // 128               # 4 contraction chunks
MCH = B // 128                 # 8 batch chunks

_NC = None
_SCALES = {}

# bf16-path score values are ~1e-5 — subnormal in fp16.  Pre-scaling ans by
# 2**16 on the host puts the device-side scores in fp16's normal range; the
# host unscales.  The fp8 path has its own scales (s_a, s_e) chosen at prep
# time so quantized inputs sit in e4m3's range and scores fit fp16.
OUT_SCALE = 2.0 ** 16


def _build_nc():
    import concourse.bacc as bacc
    import concourse.bass as bass
    import concourse.tile as tile
    from concourse import mybir

    ts, ds = bass.ts, bass.ds
    bf16 = mybir.dt.bfloat16
    f16 = mybir.dt.float16
    f8 = mybir.dt.float8e4
    f32 = mybir.dt.float32
    DR = mybir.MatmulPerfMode.DoubleRow

    nc = bacc.Bacc("TRN2", target_bir_lowering=False, debug=False)
    ansT = nc.dram_tensor("ansT", [EMB, B], bf16, kind="ExternalInput")
    ans8 = nc.dram_tensor("ans8", [128, KCH, B], f8, kind="ExternalInput")
    entT = nc.dram_tensor("entT", [EMB, NB], bf16, kind="ExternalInput")
    ent8 = nc.dram_tensor("ent8", [128, KCH, NF], f8, kind="ExternalInput")
    score = nc.dram_tensor("score", [B, SHARD_PAD], f16, kind="ExternalOutput")

    with tile.TileContext(nc) as tc:
        with tc.tile_pool(name="entp", bufs=3 * KCH) as ent_pool, \
             tc.tile_pool(name="outp", bufs=4) as out_pool, \
             tc.tile_pool(name="out8p", bufs=3) as out8_pool, \
             tc.tile_pool(name="ps", bufs=8, space="PSUM") as psum_pool:

            _frees = []
            ansT_sb, _f = tc.tile([128, KCH, B], bf16, name="ansT_sb")
            _frees.append(_f)
            ans8_sb, _f = tc.tile([128, KCH, B], f8, name="ans8_sb")
            _frees.append(_f)
            ent8_sb, _f = tc.tile([128, KCH, NF], f8, name="ent8_sb")
            _frees.append(_f)
            wup, _f = tc.tile([128, 640], bf16, name="wup")
            _frees.append(_f)

            # PE pre-warm: ~4us of zero matmuls so the HAM clock gate opens
            # during the preamble/DMA wait; real matmuls then run at 2.4 GHz
            # from the first instruction.
            nc.gpsimd.memset(wup[:], 0)
            wps = psum_pool.tile([128, NTILE], f32, name="pst")
            for i in range(10):
                nc.tensor.matmul(wps[:], wup[:, ds(0, 128)],
                                 wup[:, ds(128, 512)],
                                 start=(i == 0), stop=(i == 9))

            # inputs ride the ACT HWDGE ring (nc.scalar) — it comes out of
            # the preamble ~2.5us before the SP ring and keeps prefetches
            # from queueing behind score-output DMAs; outputs ride nc.sync
            def load_group(g, gcol):
                # one tile per k-chunk so a matmul only waits for its own DMA
                gn = GROUPS[g] * NTILE
                tiles = []
                for k in range(KCH):
                    t = ent_pool.tile([128, GN_FULL], bf16, name="ent_sb")
                    nc.scalar.dma_start(t[:, ds(0, gn)],
                                        entT[ts(k, 128), ds(gcol, gn)])
                    tiles.append(t)
                return tiles

            # startup: dma_start issue costs ~650ns of sequencer time apiece,
            # so use few, large DMAs.  k-slab order matches the first block's
            # k-outer consume order: the first matmul waits only for
            # ansT[k0] + the k0 ent slab (~0.8 MB).
            ent_sb0 = [ent_pool.tile([128, GN_FULL], bf16, name="ent_sb")
                       for _ in range(KCH)]
            gn0 = GROUPS[0] * NTILE
            nc.scalar.dma_start(ansT_sb[:, 0], ansT[ts(0, 128), :])
            nc.scalar.dma_start(ent_sb0[0][:, ds(0, gn0)],
                                entT[ts(0, 128), ds(0, gn0)])
            for k in range(1, KCH):
                nc.scalar.dma_start(ansT_sb[:, k], ansT[ts(k, 128), :])
                nc.scalar.dma_start(ent_sb0[k][:, ds(0, gn0)],
                                    entT[ts(k, 128), ds(0, gn0)])

            # gpsimd (Pool) cannot read PSUM on TRN2 — copyback on DVE + Act
            copy_engines = [nc.vector, nc.scalar]
            ci = 0

            def copyback(dst, ps):
                nonlocal ci
                eng = copy_engines[ci % len(copy_engines)]
                ci += 1
                if eng is nc.scalar:
                    eng.copy(dst, ps)
                else:
                    eng.tensor_copy(out=dst, in_=ps)

            ent_tiles = {0: ent_sb0}
            gcols = np.cumsum([0] + [gs * NTILE for gs in GROUPS]).tolist()

            def load_fp8():
                nc.scalar.dma_start(ans8_sb[:], ans8[:, :, :])
                nc.scalar.dma_start(ent8_sb[:, ds(0, 2)], ent8[:, ds(0, 2), :])
                nc.scalar.dma_start(ent8_sb[:, ds(2, 2)], ent8[:, ds(2, 2), :])

            def bf16_group(g, warm=False, last=False):
                gsz = GROUPS[g]
                gn = gsz * NTILE
                col = gcols[g]
                ent_sb = ent_tiles.pop(g)

                if warm:
                    # warm-up: k-outer with m0+m1 interleaved (2*gsz = 8 psum
                    # banks) so each k ent slab feeds 8 matmuls (~1.8us) —
                    # faster than the ~1.5us the next slab's DMA takes, so
                    # the PE never starves while group 0 lands
                    outs = [out_pool.tile([128, GN_FULL], f16, name="out_sb")
                            for _ in range(2)]
                    pss0 = [[psum_pool.tile([128, NTILE], f32, name="pst")
                             for _ in range(gsz)] for _ in range(2)]
                    for k in range(KCH):
                        for m in range(2):
                            lhsT = ansT_sb[:, k, ts(m, 128)]
                            for t in range(gsz):
                                nc.tensor.matmul(
                                    pss0[m][t][:], lhsT,
                                    ent_sb[k][:, ts(t, NTILE)],
                                    start=(k == 0), stop=(k == KCH - 1))
                                if k == KCH - 1:
                                    copyback(outs[m][:, ts(t, NTILE)],
                                             pss0[m][t][:])
                    h0 = (gsz // 2) * NTILE
                    for m in range(2):
                        nc.sync.dma_start(score[ts(m, 128), ds(col, h0)],
                                          outs[m][:, ds(0, h0)])
                        nc.sync.dma_start(score[ts(m, 128), ds(col + h0, gn - h0)],
                                          outs[m][:, ds(h0, gn - h0)])
                    ms = range(2, MCH)
                else:
                    ms = range(MCH)

                for m in ms:
                    pss = [psum_pool.tile([128, NTILE], f32, name="pst")
                           for _ in range(gsz)]
                    out_sb = out_pool.tile([128, GN_FULL], f16, name="out_sb")
                    # k outer: keeps the PE streaming one ent tile after
                    # another with the same weight chunk
                    for k in range(KCH):
                        lhsT = ansT_sb[:, k, ts(m, 128)]
                        for t in range(gsz):
                            nc.tensor.matmul(
                                pss[t][:], lhsT, ent_sb[k][:, ts(t, NTILE)],
                                start=(k == 0), stop=(k == KCH - 1))
                    for t in range(gsz):
                        copyback(out_sb[:, ts(t, NTILE)], pss[t][:])
                    if last and m == MCH - 1:
                        # fine-grained final drain: the kernel's tail is the
                        # last DMA's completion receipt, keep it small
                        for t in range(gsz - 1):
                            nc.sync.dma_start(
                                score[ts(m, 128), ds(col + t * NTILE, NTILE)],
                                out_sb[:, ds(t * NTILE, NTILE)])
                        base = (gsz - 1) * NTILE
                        nc.sync.dma_start(score[ts(m, 128), ds(col + base, 256)],
                                          out_sb[:, ds(base, 256)])
                        nc.sync.dma_start(
                            score[ts(m, 128), ds(col + base + 256, 256)],
                            out_sb[:, ds(base + 256, 256)])
                    elif gsz >= 4:
                        # two half-width output DMAs so the drain starts as
                        # soon as the first copies land
                        h0 = (gsz // 2 + 1) * NTILE
                        nc.sync.dma_start(score[ts(m, 128), ds(col, h0)],
                                          out_sb[:, ds(0, h0)])
                        nc.sync.dma_start(
                            score[ts(m, 128), ds(col + h0, gn - h0)],
                            out_sb[:, ds(h0, gn - h0)])
                    else:
                        nc.sync.dma_start(score[ts(m, 128), ds(col, gn)],
                                          out_sb[:, ds(0, gn)])

            def fp8_section():
                # fp8 DoubleRow: K=512 as 2 matmuls of 256 (2 fp8/cell).
                # Runs mid-kernel: its score writes come at 2x the bf16 rate
                # (~300 GB/s), so the surrounding bf16 groups' write slack
                # absorbs the burst instead of stretching the kernel tail.
                col8 = gcols[-1]
                for m in range(MCH):
                    out_sb = out8_pool.tile([128, NF], f16, name="out8_sb")
                    for t in range(NF_TILES):
                        ps = psum_pool.tile([128, NTILE], f32, name="pst")
                        for j in range(2):
                            nc.tensor.matmul(
                                ps[:],
                                ans8_sb[:, ds(2 * j, 2), ts(m, 128)],
                                ent8_sb[:, ds(2 * j, 2), ds(t * NTILE, NTILE)],
                                start=(j == 0), stop=(j == 1),
                                perf_mode=DR)
                        copyback(out_sb[:, ts(t, NTILE)], ps[:])
                    h0 = 6 * NTILE
                    nc.sync.dma_start(score[ts(m, 128), ds(col8, h0)],
                                      out_sb[:, ds(0, h0)])
                    nc.sync.dma_start(score[ts(m, 128), ds(col8 + h0, NF - h0)],
                                      out_sb[:, ds(h0, NF - h0)])

            # process order: b0..b5, fp8, b6 — each section's inputs are
            # issued one section ahead on the ACT ring.  fp8 sits second to
            # last: during its span only b6's small load (1 MB) competes with
            # its 2x-rate score writes (325 GB/s total, under the 358 limit),
            # and the b6 epilogue absorbs the write backlog so the kernel
            # tail stays short.
            ent_tiles[1] = load_group(1, gcols[1])
            bf16_group(0, warm=True)
            ent_tiles[2] = load_group(2, gcols[2])
            bf16_group(1)
            ent_tiles[3] = load_group(3, gcols[3])
            bf16_group(2)
            ent_tiles[4] = load_group(4, gcols[4])
            bf16_group(3)
            ent_tiles[5] = load_group(5, gcols[5])
            bf16_group(4)
            load_fp8()
            bf16_group(5)
            ent_tiles[6] = load_group(6, gcols[6])
            fp8_section()
            bf16_group(6, last=True)
            for _f in reversed(_frees):
                _f()
    nc.compile()
    return nc


def _get_nc():
    global _NC
    if _NC is None:
        _NC = _build_nc()
    return _NC


def _pmap(fn, n):
    from concurrent.futures import ThreadPoolExecutor
    with ThreadPoolExecutor(max_workers=n) as ex:
        list(ex.map(fn, range(n)))


def _to_f8_chunks(mat_t, ncols):
    """[EMB, ncols] f32 (already scaled) -> [128, KCH, ncols] e4m3 bytes."""
    q = mat_t.astype(ml_dtypes.float8_e4m3fn)
    return np.ascontiguousarray(q.reshape(KCH, 128, ncols).transpose(1, 0, 2))


def prepare_in_maps(triples, ent_emb, rel_emb):
    triples = np.asarray(triples)
    ent_emb = np.asarray(ent_emb, dtype=np.float32)
    rel_emb = np.asarray(rel_emb, dtype=np.float32)

    d = EMB // 2
    h = ent_emb[triples[:, 0].astype(np.int64)]
    r = rel_emb[triples[:, 1].astype(np.int64)]
    re_h, im_h = h[:, :d], h[:, d:]
    re_r, im_r = r[:, :d], r[:, d:]
    ans = np.empty((B, EMB), np.float32)
    ans[:, :d] = re_h * re_r - im_h * im_r
    ans[:, d:] = re_h * im_r + im_h * re_r

    ansT_bf = np.ascontiguousarray(ans.T * np.float32(OUT_SCALE)).astype(
        ml_dtypes.bfloat16)

    # fp8 scales: map absmax to ~120 (TRN e4m3 max 240), then cap the product
    # so the Cauchy-Schwarz bound on device-side scores stays inside fp16
    f8_rows = np.concatenate([
        ent_emb[c * SHARD + NB:(c + 1) * SHARD] for c in range(NCORES)])
    amax_a = float(np.abs(ans).max())
    amax_e = float(np.abs(f8_rows).max())
    s_a = 120.0 / amax_a
    s_e = 120.0 / amax_e
    cs = float(np.sqrt((ans * ans).sum(1).max()) *
               np.sqrt((f8_rows * f8_rows).sum(1).max()))
    cap = 58000.0 / cs
    if s_a * s_e > cap:
        s_a = cap / s_e
    _SCALES["fp8_inv"] = 1.0 / (s_a * s_e)

    ans8 = _to_f8_chunks(np.ascontiguousarray(ans.T) * np.float32(s_a), B)

    ent_bf = np.empty((NCORES, EMB, NB), dtype=ml_dtypes.bfloat16)
    ent8s = np.empty((NCORES, 128, KCH, NF), dtype=ml_dtypes.float8_e4m3fn)

    def _core(c):
        rows = ent_emb[c * SHARD:(c + 1) * SHARD]
        ent_bf[c] = rows[:NB].T
        blk = np.zeros((EMB, NF), np.float32)
        blk[:, :NF_REAL] = rows[NB:].T * np.float32(s_e)
        ent8s[c] = _to_f8_chunks(blk, NF)

    _pmap(_core, NCORES)
    return [{"ansT": ansT_bf, "ans8": ans8, "entT": ent_bf[c],
             "ent8": ent8s[c]} for c in range(NCORES)]


def run_raw(in_maps, trace=False):
    from concourse import bass_utils
    return bass_utils.run_bass_kernel_spmd(
        _get_nc(), in_maps, core_ids=list(range(NCORES)), trace=trace
    )


def assemble(results):
    out = np.empty((B, NUM_ENT), np.float32)
    inv16 = np.float32(1.0 / OUT_SCALE)
    inv8 = np.float32(_SCALES["fp8_inv"])

    def _one(c):
        sh = results[c]["score"]
        bf = sh[:, :NB].astype(np.float32)
        bf *= inv16
        f8 = sh[:, NB:NB + NF_REAL].astype(np.float32)
        f8 *= inv8
        out[:, c * SHARD:c * SHARD + NB] = bf
        out[:, c * SHARD + NB:(c + 1) * SHARD] = f8

    _pmap(_one, NCORES)
    return out


def kernel(triples, ent_emb, rel_emb):
    in_maps = prepare_in_maps(triples, ent_emb, rel_emb)
    res = run_raw(in_maps)
    return assemble(res.results)


# revision 20
# speedup vs baseline: 1.0091x; 1.0091x over previous
"""ComplEx scoring kernel for 8 Trainium2 NeuronCores.

Math: score[b, e] = Re(<h_b * r_b, conj(ent_e)>) with h = ent_emb[triples[:,0]],
r = rel_emb[triples[:,1]].  Writing ans_b = concat(re_h*re_r - im_h*im_r,
re_h*im_r + im_h*re_r) (shape [B, 512]), the score is exactly
score = ans @ ent_emb.T  — one [1024, 512] x [512, 200000] GEMM.

Strategy (vocab/tensor parallel on the entity axis, 25000 entities/core,
padded to 25088 = 49x512 columns):
  - host: tiny gather + complex multiply -> ans  (microseconds)
  - the GEMM is TensorE-bound (26.3 GFLOP/core vs 78.6 TF/s bf16), so the
    entity axis is split into a bf16 part (36 tiles of 512) and an fp8-e4m3
    DoubleRow part (13 tiles) that runs the PE at 2 MACs/cell/cycle.  The
    fp8 fraction (26% of entities) is sized so the fp8 quantization noise
    (3.75e-2 on those columns) keeps the global rel err at ~1.93e-2 (<2e-2).
  - per core: score_bf16[1024, 18432] + score_fp8[1024, 6656], both f16.
    PE is pre-warmed with dummy matmuls so the HAM clock gate opens during
    the preamble/first DMAs instead of 3.4us into real work.  Inputs ride
    the ACT HWDGE ring, outputs the SP ring.  The fp8 section runs second
    to last: its score writes come at 2x the bf16 rate (~300 GB/s), and the
    small final bf16 group absorbs the write backlog so the kernel tail is
    just the last small DMA's completion receipt.
  - host: concatenate the 8 column slabs, per-region unscale, drop padding
"""

import numpy as np
import ml_dtypes

NCORES = 8
NUM_ENT = 200000
EMB = 512
B = 1024
SHARD = NUM_ENT // NCORES      # 25000 entities per core
NTILE = 512                    # matmul moving free dim == one PSUM bank
NB_TILES = 36                  # bf16 512-tiles per core
NF_TILES = 13                  # fp8 512-tiles per core
NB = NB_TILES * NTILE          # 18432 bf16 columns
NF = NF_TILES * NTILE          # 6656 fp8 columns (6568 real + 88 pad)
SHARD_PAD = NB + NF            # 25088
NF_REAL = SHARD - NB           # 6568 real entities in the fp8 region
GROUPS = [4, 4, 7, 7, 7, 5, 2] # bf16 tile groups (DMA/reuse granularity)
GN_FULL = 7 * NTILE            # 3584
KCH = EMB // 128               # 4 contraction chunks
MCH = B // 128                 # 8 batch chunks

_NC = None
_SCALES = {}

# bf16-path score values are ~1e-5 — subnormal in fp16.  Pre-scaling ans by
# 2**16 on the host puts the device-side scores in fp16's normal range; the
# host unscales.  The fp8 path has its own scales (s_a, s_e) chosen at prep
# time so quantized inputs sit in e4m3's range and scores fit fp16.
OUT_SCALE = 2.0 ** 16


def _build_nc():
    import concourse.bacc as bacc
    import concourse.bass as bass
    import concourse.tile as tile
    from concourse import mybir

    ts, ds = bass.ts, bass.ds
    bf16 = mybir.dt.bfloat16
    f16 = mybir.dt.float16
    f8 = mybir.dt.float8e4
    f32 = mybir.dt.float32
    DR = mybir.MatmulPerfMode.DoubleRow

    nc = bacc.Bacc("TRN2", target_bir_lowering=False, debug=False)
    ansT = nc.dram_tensor("ansT", [EMB, B], bf16, kind="ExternalInput")
    ans8 = nc.dram_tensor("ans8", [128, KCH, B], f8, kind="ExternalInput")
    entT = nc.dram_tensor("entT", [EMB, NB], bf16, kind="ExternalInput")
    ent8 = nc.dram_tensor("ent8", [128, KCH, NF], f8, kind="ExternalInput")
    score = nc.dram_tensor("score", [B, SHARD_PAD], f16, kind="ExternalOutput")

    with tile.TileContext(nc) as tc:
        with tc.tile_pool(name="entp", bufs=3 * KCH) as ent_pool, \
             tc.tile_pool(name="outp", bufs=4) as out_pool, \
             tc.tile_pool(name="out8p", bufs=3) as out8_pool, \
             tc.tile_pool(name="ps", bufs=8, space="PSUM") as psum_pool:

            _frees = []
            ansT_sb, _f = tc.tile([128, KCH, B], bf16, name="ansT_sb")
            _frees.append(_f)
            ans8_sb, _f = tc.tile([128, KCH, B], f8, name="ans8_sb")
            _frees.append(_f)
            ent8_sb, _f = tc.tile([128, KCH, NF], f8, name="ent8_sb")
            _frees.append(_f)
            wup, _f = tc.tile([128, 640], bf16, name="wup")
            _frees.append(_f)

            # PE pre-warm: ~4us of zero matmuls so the HAM clock gate opens
            # during the preamble/DMA wait; real matmuls then run at 2.4 GHz
            # from the first instruction.
            nc.gpsimd.memset(wup[:], 0)
            wps = psum_pool.tile([128, NTILE], f32, name="pst")
            for i in range(12):
                nc.tensor.matmul(wps[:], wup[:, ds(0, 128)],
                                 wup[:, ds(128, 512)],
                                 start=(i == 0), stop=(i == 11))

            # inputs ride the ACT HWDGE ring (nc.scalar) — it comes out of
            # the preamble ~2.5us before the SP ring and keeps prefetches
            # from queueing behind score-output DMAs; outputs ride nc.sync
            def load_group(g, gcol):
                # one tile per k-chunk so a matmul only waits for its own DMA
                gn = GROUPS[g] * NTILE
                tiles = []
                for k in range(KCH):
                    t = ent_pool.tile([128, GN_FULL], bf16, name="ent_sb")
                    nc.scalar.dma_start(t[:, ds(0, gn)],
                                        entT[ts(k, 128), ds(gcol, gn)])
                    tiles.append(t)
                return tiles

            # startup: dma_start issue costs ~650ns of sequencer time apiece,
            # so use few, large DMAs.  k-slab order matches the first block's
            # k-outer consume order: the first matmul waits only for
            # ansT[k0] + the k0 ent slab (~0.8 MB).
            ent_sb0 = [ent_pool.tile([128, GN_FULL], bf16, name="ent_sb")
                       for _ in range(KCH)]
            gn0 = GROUPS[0] * NTILE
            nc.scalar.dma_start(ansT_sb[:, 0], ansT[ts(0, 128), :])
            nc.scalar.dma_start(ent_sb0[0][:, ds(0, gn0)],
                                entT[ts(0, 128), ds(0, gn0)])
            for k in range(1, KCH):
                nc.scalar.dma_start(ansT_sb[:, k], ansT[ts(k, 128), :])
                nc.scalar.dma_start(ent_sb0[k][:, ds(0, gn0)],
                                    entT[ts(k, 128), ds(0, gn0)])

            # gpsimd (Pool) cannot read PSUM on TRN2 — copyback on DVE + Act
            copy_engines = [nc.vector, nc.scalar]
            ci = 0

            def copyback(dst, ps):
                nonlocal ci
                eng = copy_engines[ci % len(copy_engines)]
                ci += 1
                if eng is nc.scalar:
                    eng.copy(dst, ps)
                else:
                    eng.tensor_copy(out=dst, in_=ps)

            ent_tiles = {0: ent_sb0}
            gcols = np.cumsum([0] + [gs * NTILE for gs in GROUPS]).tolist()

            def load_fp8():
                nc.scalar.dma_start(ans8_sb[:], ans8[:, :, :])
                nc.scalar.dma_start(ent8_sb[:, ds(0, 2)], ent8[:, ds(0, 2), :])
                nc.scalar.dma_start(ent8_sb[:, ds(2, 2)], ent8[:, ds(2, 2), :])

            def bf16_group(g, warm=False, last=False):
                gsz = GROUPS[g]
                gn = gsz * NTILE
                col = gcols[g]
                ent_sb = ent_tiles.pop(g)

                if warm:
                    # warm-up: k-outer with m0+m1 interleaved (2*gsz = 8 psum
                    # banks) so each k ent slab feeds 8 matmuls (~1.8us) —
                    # faster than the ~1.5us the next slab's DMA takes, so
                    # the PE never starves while group 0 lands
                    outs = [out_pool.tile([128, GN_FULL], f16, name="out_sb")
                            for _ in range(2)]
                    pss0 = [[psum_pool.tile([128, NTILE], f32, name="pst")
                             for _ in range(gsz)] for _ in range(2)]
                    for k in range(KCH):
                        for m in range(2):
                            lhsT = ansT_sb[:, k, ts(m, 128)]
                            for t in range(gsz):
                                nc.tensor.matmul(
                                    pss0[m][t][:], lhsT,
                                    ent_sb[k][:, ts(t, NTILE)],
                                    start=(k == 0), stop=(k == KCH - 1))
                                if k == KCH - 1:
                                    copyback(outs[m][:, ts(t, NTILE)],
                                             pss0[m][t][:])
                    h0 = (gsz // 2) * NTILE
                    for m in range(2):
                        nc.sync.dma_start(score[ts(m, 128), ds(col, h0)],
                                          outs[m][:, ds(0, h0)])
                        nc.sync.dma_start(score[ts(m, 128), ds(col + h0, gn - h0)],
                                          outs[m][:, ds(h0, gn - h0)])
                    ms = range(2, MCH)
                else:
                    ms = range(MCH)

                for m in ms:
                    pss = [psum_pool.tile([128, NTILE], f32, name="pst")
                           for _ in range(gsz)]
                    out_sb = out_pool.tile([128, GN_FULL], f16, name="out_sb")
                    # k outer: keeps the PE streaming one ent tile after
                    # another with the same weight chunk
                    for k in range(KCH):
                        lhsT = ansT_sb[:, k, ts(m, 128)]
                        for t in range(gsz):
                            nc.tensor.matmul(
                                pss[t][:], lhsT, ent_sb[k][:, ts(t, NTILE)],
                                start=(k == 0), stop=(k == KCH - 1))
                    for t in range(gsz):
                        copyback(out_sb[:, ts(t, NTILE)], pss[t][:])
                    if last and m == MCH - 1:
                        # fine-grained final drain: the kernel's tail is the
                        # last DMA's completion receipt, keep it small
                        for t in range(gsz - 1):
                            nc.sync.dma_start(
                                score[ts(m, 128), ds(col + t * NTILE, NTILE)],
                                out_sb[:, ds(t * NTILE, NTILE)])
                        base = (gsz - 1) * NTILE
                        nc.sync.dma_start(score[ts(m, 128), ds(col + base, 256)],
                                          out_sb[:, ds(base, 256)])
                        nc.sync.dma_start(
                            score[ts(m, 128), ds(col + base + 256, 256)],
                            out_sb[:, ds(base + 256, 256)])
                    elif gsz >= 4:
                        # two half-width output DMAs so the drain starts as
                        # soon as the first copies land
                        h0 = (gsz // 2 + 1) * NTILE
                        nc.sync.dma_start(score[ts(m, 128), ds(col, h0)],
                                          out_sb[:, ds(0, h0)])
                        nc.sync.dma_start(
                            score[ts(m, 128), ds(col + h0, gn - h0)],
                            out_sb[:, ds(h0, gn - h0)])
                    else:
                        nc.sync.dma_start(score[ts(m, 128), ds(col, gn)],
                                          out_sb[:, ds(0, gn)])

            def fp8_section():
                # fp8 DoubleRow: K=512 as 2 matmuls of 256 (2 fp8/cell).
                # Runs mid-kernel: its score writes come at 2x the bf16 rate
                # (~300 GB/s), so the surrounding bf16 groups' write slack
                # absorbs the burst instead of stretching the kernel tail.
                col8 = gcols[-1]
                for m in range(MCH):
                    out_sb = out8_pool.tile([128, NF], f16, name="out8_sb")
                    for t in range(NF_TILES):
                        ps = psum_pool.tile([128, NTILE], f32, name="pst")
                        for j in range(2):
                            nc.tensor.matmul(
                                ps[:],
                                ans8_sb[:, ds(2 * j, 2), ts(m, 128)],
                                ent8_sb[:, ds(2 * j, 2), ds(t * NTILE, NTILE)],
                                start=(j == 0), stop=(j == 1),
                                perf_mode=DR)
                        copyback(out_sb[:, ts(t, NTILE)], ps[:])
                    h0 = 6 * NTILE
                    nc.sync.dma_start(score[ts(m, 128), ds(col8, h0)],
                                      out_sb[:, ds(0, h0)])
                    nc.sync.dma_start(score[ts(m, 128), ds(col8 + h0, NF - h0)],
                                      out_sb[:, ds(h0, NF - h0)])

            # process order: b0..b5, fp8, b6 — each section's inputs are
            # issued one section ahead on the ACT ring.  fp8 sits second to
            # last: during its span only b6's small load (1 MB) competes with
            # its 2x-rate score writes (325 GB/s total, under the 358 limit),
            # and the b6 epilogue absorbs the write backlog so the kernel
            # tail stays short.
            ent_tiles[1] = load_group(1, gcols[1])
            bf16_group(0, warm=True)
            ent_tiles[2] = load_group(2, gcols[2])
            bf16_group(1)
            ent_tiles[3] = load_group(3, gcols[3])
            bf16_group(2)
            ent_tiles[4] = load_group(4, gcols[4])
            bf16_group(3)
            ent_tiles[5] = load_group(5, gcols[5])
            bf16_group(4)
            load_fp8()
            bf16_group(5)
            ent_tiles[6] = load_group(6, gcols[6])
            fp8_section()
            bf16_group(6, last=True)
            for _f in reversed(_frees):
                _f()
    nc.compile()
    return nc


def _get_nc():
    global _NC
    if _NC is None:
        _NC = _build_nc()
    return _NC


def _pmap(fn, n):
    from concurrent.futures import ThreadPoolExecutor
    with ThreadPoolExecutor(max_workers=n) as ex:
        list(ex.map(fn, range(n)))


def _to_f8_chunks(mat_t, ncols):
    """[EMB, ncols] f32 (already scaled) -> [128, KCH, ncols] e4m3 bytes."""
    q = mat_t.astype(ml_dtypes.float8_e4m3fn)
    return np.ascontiguousarray(q.reshape(KCH, 128, ncols).transpose(1, 0, 2))


def prepare_in_maps(triples, ent_emb, rel_emb):
    triples = np.asarray(triples)
    ent_emb = np.asarray(ent_emb, dtype=np.float32)
    rel_emb = np.asarray(rel_emb, dtype=np.float32)

    d = EMB // 2
    h = ent_emb[triples[:, 0].astype(np.int64)]
    r = rel_emb[triples[:, 1].astype(np.int64)]
    re_h, im_h = h[:, :d], h[:, d:]
    re_r, im_r = r[:, :d], r[:, d:]
    ans = np.empty((B, EMB), np.float32)
    ans[:, :d] = re_h * re_r - im_h * im_r
    ans[:, d:] = re_h * im_r + im_h * re_r

    ansT_bf = np.ascontiguousarray(ans.T * np.float32(OUT_SCALE)).astype(
        ml_dtypes.bfloat16)

    # fp8 scales: map absmax to ~120 (TRN e4m3 max 240), then cap the product
    # so the Cauchy-Schwarz bound on device-side scores stays inside fp16
    f8_rows = np.concatenate([
        ent_emb[c * SHARD + NB:(c + 1) * SHARD] for c in range(NCORES)])
    amax_a = float(np.abs(ans).max())
    amax_e = float(np.abs(f8_rows).max())
    s_a = 120.0 / amax_a
    s_e = 120.0 / amax_e
    cs = float(np.sqrt((ans * ans).sum(1).max()) *
               np.sqrt((f8_rows * f8_rows).sum(1).max()))
    cap = 58000.0 / cs
    if s_a * s_e > cap:
        s_a = cap / s_e
    _SCALES["fp8_inv"] = 1.0 / (s_a * s_e)

    ans8 = _to_f8_chunks(np.ascontiguousarray(ans.T) * np.float32(s_a), B)

    ent_bf = np.empty((NCORES, EMB, NB), dtype=ml_dtypes.bfloat16)
    ent8s = np.empty((NCORES, 128, KCH, NF), dtype=ml_dtypes.float8_e4m3fn)

    def _core(c):
        rows = ent_emb[c * SHARD:(c + 1) * SHARD]
        ent_bf[c] = rows[:NB].T
        blk = np.zeros((EMB, NF), np.float32)
        blk[:, :NF_REAL] = rows[NB:].T * np.float32(s_e)
        ent8s[c] = _to_f8_chunks(blk, NF)

    _pmap(_core, NCORES)
    return [{"ansT": ansT_bf, "ans8": ans8, "entT": ent_bf[c],
             "ent8": ent8s[c]} for c in range(NCORES)]


def run_raw(in_maps, trace=False):
    from concourse import bass_utils
    return bass_utils.run_bass_kernel_spmd(
        _get_nc(), in_maps, core_ids=list(range(NCORES)), trace=trace
    )


def assemble(results):
    out = np.empty((B, NUM_ENT), np.float32)
    inv16 = np.float32(1.0 / OUT_SCALE)
    inv8 = np.float32(_SCALES["fp8_inv"])

    def _one(c):
        sh = results[c]["score"]
        bf = sh[:, :NB].astype(np.float32)
        bf *= inv16
        f8 = sh[:, NB:NB + NF_REAL].astype(np.float32)
        f8 *= inv8
        out[:, c * SHARD:c * SHARD + NB] = bf
        out[:, c * SHARD + NB:(c + 1) * SHARD] = f8

    _pmap(_one, NCORES)
    return out


def kernel(triples, ent_emb, rel_emb):
    in_maps = prepare_in_maps(triples, ent_emb, rel_emb)
    res = run_raw(in_maps)
    return assemble(res.results)
